# revision 8
# baseline (speedup 1.0000x reference)
"""Bass/Trainium2 kernel for nn_AStarScanStrategy (scatter_memory).

Math simplification: the reference gathers feat_hw[idx[n]], applies a linear
map, and scatter-adds the result back to bin idx[n], then divides by the
count. Every value accumulated into bin hw is identical
(feat_hw[hw] @ W_m + b_m), so after the divide the output is exactly

    out[b, :, hw] = (W_m^T @ feat[b, :, hw] + b_m) * occupancy(b, hw)

where occupancy(b, hw) = 1 if hw appears in path_idx[b], else 0.

Device kernel (data-parallel over batch, 2 batches/core on 8 cores): the two
batches are stacked on the 128 SBUF partitions (channels 0:64 = batch A,
64:128 = batch B) so every engine runs full-width:

  - psum = W2^T @ feat_pair with W2 = blockdiag(W_m, W_m), bf16 in/out.
  - occupancy mask bytes (fp8 0/1, host-computed support set — the host
    already owned the dedup in the scatter formulation) are loaded via
    64-way partition-broadcast DMA and applied in the PSUM->SBUF drain:
    one DVE tensor_tensor multiply per 1024-col group, bf16 output.
  - columns are processed in graduated chunks (3072 at head/tail, 6144
    mid) so the first chunk's deps land early and the tail store is short.
  - per chunk j: feat and mask loads go to OPPOSITE HWDGE rings (sync /
    scalar alternating by parity) in chunk order, so both deps of a chunk
    land in parallel and every load precedes every store in ring-FIFO
    order; stores alternate rings behind the loads.

Host folds b_m in as out += outer(b_m, mask) per batch (b_m is zeros for
this problem, so the branch is normally skipped) and upcasts bf16 -> f32.
"""

import sys

if "/opt/trn_rl_repo" not in sys.path:
    sys.path.insert(0, "/opt/trn_rl_repo")

import numpy as np

# Problem constants (hardcoded; kernel.py must be self-contained).
B, C, H, W = 16, 64, 192, 192
HW = H * W  # 36864
P, L = 128, 512
NCORES = 8
BPC = B // NCORES  # batches per core = 2

WIDTHS = [3072, 3072, 6144, 6144, 6144, 6144, 3072, 3072]  # sum = HW
DG = 1024  # columns per PSUM tile / DVE drain group

FP8_ONE = 0x38  # float8e4 encoding of 1.0

_CACHE: dict = {}


def _build():
    import concourse.mybir as mybir
    import concourse.tile as tile
    from concourse import bacc

    F32 = mybir.dt.float32
    FP8 = mybir.dt.float8e4
    BF16 = mybir.dt.bfloat16
    U8 = mybir.dt.uint8

    nc = bacc.Bacc(None, target_bir_lowering=False, debug=False)

    feat_ext = nc.dram_tensor("featpair", [128, HW], BF16, kind="ExternalInput")
    mask_ext = nc.dram_tensor("maskbytes", [BPC, 1, HW], U8, kind="ExternalInput")
    w2_ext = nc.dram_tensor("W2", [128, 128], BF16, kind="ExternalInput")
    out_ext = nc.dram_tensor("outpair", [128, HW], BF16, kind="ExternalOutput")

    offs = np.cumsum([0] + WIDTHS).tolist()

    with tile.TileContext(nc) as tc:
        with (
            tc.tile_pool(name="const", bufs=1) as const,
            tc.tile_pool(name="feat", bufs=1) as featp,
            tc.tile_pool(name="outp", bufs=3) as outp,
            tc.tile_pool(name="maskp", bufs=1) as maskp,
            tc.tile_pool(name="psum", bufs=4, space="PSUM") as psum,
        ):
            w2 = const.tile([128, 128], BF16)
            nc.scalar.dma_start(out=w2[:], in_=w2_ext[:])

            # chunk-ordered loads; feat and mask of one chunk on opposite
            # rings; all loads precede all stores within each ring
            fts, mts = [], []
            for j, w in enumerate(WIDTHS):
                c0 = offs[j]
                f_eng = nc.sync if j % 2 == 0 else nc.scalar
                m_eng = nc.scalar if j % 2 == 0 else nc.sync
                ft = featp.tile([128, w], BF16, tag=f"f{j}", name=f"ft{j}")
                f_eng.dma_start(out=ft[:], in_=feat_ext[:, c0 : c0 + w])
                mt = maskp.tile([128, w], FP8, tag=f"m{j}", name=f"mt{j}")
                for half in range(2):
                    m_eng.dma_start(
                        out=mt[64 * half : 64 * (half + 1), :],
                        in_=mask_ext[half, :, c0 : c0 + w]
                        .bitcast(FP8)
                        .partition_broadcast(64),
                    )
                fts.append(ft)
                mts.append(mt)

            for j, w in enumerate(WIDTHS):
                ft, mt = fts[j], mts[j]
                ot = outp.tile([128, w], BF16, tag="ot", name=f"ot{j}")
                for g in range(w // DG):
                    pv = psum.tile([128, DG], F32, tag="pv", name=f"pv{j}_{g}")
                    for h in range(2):
                        s_in = slice(g * DG + h * 512, g * DG + (h + 1) * 512)
                        s_ps = slice(h * 512, (h + 1) * 512)
                        nc.tensor.matmul(
                            pv[:, s_ps],
                            w2[:],
                            ft[:, s_in],
                            start=True,
                            stop=True,
                        )
                    nc.vector.tensor_tensor(
                        out=ot[:, g * DG : (g + 1) * DG],
                        in0=pv[:],
                        in1=mt[:, g * DG : (g + 1) * DG],
                        op=mybir.AluOpType.mult,
                    )
                s_eng = nc.sync if j % 2 == 0 else nc.scalar
                s_eng.dma_start(
                    out=out_ext[:, offs[j] : offs[j] + w], in_=ot[:]
                )
    nc.compile()
    return nc


def _get_nc():
    if "nc" not in _CACHE:
        _CACHE["nc"] = _build()
    return _CACHE["nc"]


def _shard_inputs(features, path_idx, W_m, b_m):
    import ml_dtypes

    bf16 = ml_dtypes.bfloat16
    fb = np.asarray(features, dtype=np.float32).reshape(B, C, HW).astype(bf16)
    idx = np.asarray(path_idx).reshape(B, P * L).astype(np.int64)
    occ = np.zeros((B, HW), np.uint8)
    occ[np.arange(B)[:, None], idx] = FP8_ONE
    w = np.asarray(W_m, dtype=np.float32).astype(bf16)
    W2 = np.zeros((128, 128), bf16)
    W2[:C, :C] = w
    W2[C:, C:] = w
    in_maps = []
    for c in range(NCORES):
        bA, bB = BPC * c, BPC * c + 1
        st = np.concatenate([fb[bA], fb[bB]], axis=0)  # [128, HW]
        in_maps.append(
            {
                "featpair": np.ascontiguousarray(st),
                "maskbytes": occ[bA : bB + 1].reshape(BPC, 1, HW),
                "W2": W2,
            }
        )
    return in_maps


def kernel(features, path_idx, W_m, b_m, trace=False, **trace_kwargs):
    from concourse.bass_utils import run_bass_kernel_spmd

    nc = _get_nc()
    in_maps = _shard_inputs(features, path_idx, W_m, b_m)
    res = run_bass_kernel_spmd(
        nc, in_maps, list(range(NCORES)), trace=trace, **trace_kwargs
    )
    outs = []
    for c in range(NCORES):
        op = np.asarray(res.results[c]["outpair"])  # [128, HW] bf16
        outs.append(np.stack([op[:C], op[C:]]))
    out = np.concatenate(outs, axis=0).astype(np.float32)  # [B, C, HW]
    bm = np.asarray(b_m, dtype=np.float32).reshape(C)
    if np.any(bm != 0.0):
        idx = np.asarray(path_idx).reshape(B, P * L).astype(np.int64)
        m01 = np.zeros((B, HW), np.float32)
        m01[np.arange(B)[:, None], idx] = 1.0
        out += bm[None, :, None] * m01[:, None, :]
    out = out.reshape(B, C, H, W)
    if trace:
        _CACHE["last_result"] = res
    return out


# revision 18
# speedup vs baseline: 1.2126x; 1.2126x over previous
"""Bass/Trainium2 kernel for nn_AStarScanStrategy (scatter_memory).

Math simplification: the reference gathers feat_hw[idx[n]], applies a linear
map, and scatter-adds the result back to bin idx[n], then divides by the
count. Every value accumulated into bin hw is identical
(feat_hw[hw] @ W_m + b_m), so after the divide the output is exactly

    out[b, :, hw] = (W_m^T @ feat[b, :, hw] + b_m) * occupancy(b, hw)

where occupancy(b, hw) = 1 if hw appears in path_idx[b], else 0.

Device kernel (data-parallel over batch, 2 batches/core on 8 cores): the two
batches are stacked on the 128 SBUF partitions (channels 0:64 = batch A,
64:128 = batch B) so every engine runs full-width:

  - psum = W2^T @ feat_pair with W2 = blockdiag(W_m, W_m), bf16 in/out.
  - occupancy mask bytes (fp8 0/1, host-computed support set — the host
    already owned the dedup in the scatter formulation) are loaded via
    64-way partition-broadcast DMA and applied in the PSUM->SBUF drain:
    one DVE tensor_tensor multiply per 1024-col group, bf16 output.
  - columns are processed in graduated chunks (3072 at head/tail, 6144
    mid) so the first chunk's deps land early and the tail store is short.
  - per chunk j: feat and mask loads go to OPPOSITE HWDGE rings (sync /
    scalar alternating by parity) in chunk order, so both deps of a chunk
    land in parallel and every load precedes every store in ring-FIFO
    order; stores alternate rings behind the loads.

Host folds b_m in as out += outer(b_m, mask) per batch (b_m is zeros for
this problem, so the branch is normally skipped) and upcasts bf16 -> f32.
"""

import sys

if "/opt/trn_rl_repo" not in sys.path:
    sys.path.insert(0, "/opt/trn_rl_repo")

import numpy as np

# Problem constants (hardcoded; kernel.py must be self-contained).
B, C, H, W = 16, 64, 192, 192
HW = H * W  # 36864
P, L = 128, 512
NCORES = 8
BPC = B // NCORES  # batches per core = 2

CHUNK = 6144
NCHUNK = HW // CHUNK  # 6
MW = 2 * CHUNK  # mask tile width (2 chunks per broadcast)
DG = 1024  # columns per PSUM tile / DVE drain group
NDG = CHUNK // DG  # 6

FP8_ONE = 0x38  # float8e4 encoding of 1.0

_CACHE: dict = {}


def _build():
    import concourse.mybir as mybir
    import concourse.tile as tile
    from concourse import bacc

    F32 = mybir.dt.float32
    FP8 = mybir.dt.float8e4
    BF16 = mybir.dt.bfloat16
    U8 = mybir.dt.uint8

    nc = bacc.Bacc(None, target_bir_lowering=False, debug=False)

    feat_ext = nc.dram_tensor("featpair", [128, HW], BF16, kind="ExternalInput")
    mask_ext = nc.dram_tensor("maskrep", [128, HW], U8, kind="ExternalInput")
    w2_ext = nc.dram_tensor("W2", [128, 128], BF16, kind="ExternalInput")
    out_ext = nc.dram_tensor("outpair", [128, HW], BF16, kind="ExternalOutput")

    with tile.TileContext(nc) as tc:
        with (
            tc.tile_pool(name="const", bufs=1) as const,
            tc.tile_pool(name="feat", bufs=1) as featp,
            tc.tile_pool(name="outp", bufs=3) as outp,
            tc.tile_pool(name="maskp", bufs=1) as maskp,
            tc.tile_pool(name="psum", bufs=4, space="PSUM") as psum,
        ):
            w2 = const.tile([128, 128], BF16)
            nc.scalar.dma_start(out=w2[:], in_=w2_ext[:])
            # stage the two mask rows once (2 fat descriptors), then fan out
            # on-chip: gpsimd partition_broadcast does SBUF->SBUF replication
            # without touching HBM and without occupying the HWDGE rings.


            # All loads chunk-ordered, split across the two rings; the mask
            # (host-pre-replicated to all 128 partitions: unique DRAM rows,
            # fat descriptors, no same-address HBM hot-spotting) rides the
            # ring opposite its chunk's feat. Loads precede stores per ring.
            fts, mts = [], []
            for j in range(NCHUNK):
                ft = featp.tile([128, CHUNK], BF16, tag=f"f{j}", name=f"ft{j}")
                eng = nc.sync if j % 2 == 0 else nc.scalar
                eng.dma_start(
                    out=ft[:], in_=feat_ext[:, j * CHUNK : (j + 1) * CHUNK]
                )
                fts.append(ft)
                if j % 2 == 0:
                    c0 = j * CHUNK
                    mt = maskp.tile([128, MW], FP8, tag=f"m{j}", name=f"mt{j}")
                    nc.scalar.dma_start(
                        out=mt[:], in_=mask_ext[:, c0 : c0 + MW].bitcast(FP8)
                    )
                    mts.append(mt)

            for j in range(NCHUNK):
                ft = fts[j]
                mt = mts[j // 2]
                m0 = (j % 2) * CHUNK
                ot = outp.tile([128, CHUNK], BF16, tag="ot", name=f"ot{j}")
                for g in range(NDG):
                    pv = psum.tile([128, DG], F32, tag="pv", name=f"pv{j}_{g}")
                    for h in range(2):
                        s_in = slice(g * DG + h * 512, g * DG + (h + 1) * 512)
                        s_ps = slice(h * 512, (h + 1) * 512)
                        nc.tensor.matmul(
                            pv[:, s_ps],
                            w2[:],
                            ft[:, s_in],
                            start=True,
                            stop=True,
                        )
                    nc.vector.tensor_tensor(
                        out=ot[:, g * DG : (g + 1) * DG],
                        in0=pv[:],
                        in1=mt[:, m0 + g * DG : m0 + (g + 1) * DG],
                        op=mybir.AluOpType.mult,
                    )
                eng = nc.sync if j % 2 == 0 else nc.scalar
                eng.dma_start(
                    out=out_ext[:, j * CHUNK : (j + 1) * CHUNK], in_=ot[:]
                )
    nc.compile()
    return nc


def _get_nc():
    if "nc" not in _CACHE:
        _CACHE["nc"] = _build()
    return _CACHE["nc"]


def _shard_inputs(features, path_idx, W_m, b_m):
    import ml_dtypes

    bf16 = ml_dtypes.bfloat16
    fb = np.asarray(features, dtype=np.float32).reshape(B, C, HW).astype(bf16)
    idx = np.asarray(path_idx).reshape(B, P * L).astype(np.int64)
    occ = np.zeros((B, HW), np.uint8)
    occ[np.arange(B)[:, None], idx] = FP8_ONE
    w = np.asarray(W_m, dtype=np.float32).astype(bf16)
    W2 = np.zeros((128, 128), bf16)
    W2[:C, :C] = w
    W2[C:, C:] = w
    in_maps = []
    for c in range(NCORES):
        bA, bB = BPC * c, BPC * c + 1
        st = np.concatenate([fb[bA], fb[bB]], axis=0)  # [128, HW]
        mrep = np.empty((128, HW), np.uint8)
        mrep[:C] = occ[bA]
        mrep[C:] = occ[bB]
        in_maps.append(
            {
                "featpair": np.ascontiguousarray(st),
                "maskrep": mrep,
                "W2": W2,
            }
        )
    return in_maps


def kernel(features, path_idx, W_m, b_m, trace=False, **trace_kwargs):
    from concourse.bass_utils import run_bass_kernel_spmd

    nc = _get_nc()
    in_maps = _shard_inputs(features, path_idx, W_m, b_m)
    res = run_bass_kernel_spmd(
        nc, in_maps, list(range(NCORES)), trace=trace, **trace_kwargs
    )
    outs = []
    for c in range(NCORES):
        op = np.asarray(res.results[c]["outpair"])  # [128, HW] bf16
        outs.append(np.stack([op[:C], op[C:]]))
    out = np.concatenate(outs, axis=0).astype(np.float32)  # [B, C, HW]
    bm = np.asarray(b_m, dtype=np.float32).reshape(C)
    if np.any(bm != 0.0):
        idx = np.asarray(path_idx).reshape(B, P * L).astype(np.int64)
        m01 = np.zeros((B, HW), np.float32)
        m01[np.arange(B)[:, None], idx] = 1.0
        out += bm[None, :, None] * m01[:, None, :]
    out = out.reshape(B, C, H, W)
    if trace:
        _CACHE["last_result"] = res
    return out
